# revision 4
# baseline (speedup 1.0000x reference)
"""GNN message-passing kernel for Trainium2 (8 NeuronCores, edge-parallel).

Strategy: shard edges by source-node range (host-side stable sort), so each
core owns nodes [c*6250, (c+1)*6250) and exactly the edges rooted there.
Outputs are disjoint -> no collective needed; host concatenates.

Device pipeline per core (E_pad edges in 128-edge tiles, 49 node blocks):
  1. PE: per-edge Q/K/V projections (stationary = gathered-feature tile^T,
     moving = fused [Wq|Wk|Wv'] weight block), PSUM f32.
  2. DVE: bias-add fused into PSUM->SBUF bf16 cast.
  3. DVE/ACT: per-edge 8x8 head attention via broadcast-AP multiply +
     segmented reduce; exp on ACT.
  4. PE: segment-sum via block one-hot matmul (S matrix from host),
     plus edge-type one-hot/count columns for the embedding & bias terms.
  5. PE: final output projection [Wo^T | emb@Wo^T | bo] per node block.
"""

import os
import sys

sys.path.insert(0, "/opt/trn_rl_repo")

import numpy as np
import ml_dtypes

from concourse import bass, bacc, mybir
import concourse.tile as tile
from concourse.bass_utils import run_bass_kernel_spmd

N_NODES = 50000
N_CORES = 8
NPC = N_NODES // N_CORES  # 6250
NB = 49                   # node blocks of 128 per core (49*128 = 6272)
NODES_PAD = NB * 128
IN_DIM = 128
HID = 256
H = 8
D = 32

BF16 = ml_dtypes.bfloat16
_prog_cache = {}
LAST_RESULTS = None


def _build_program(C):
    """C = edge-chunks (of 128) per node block; T = NB*C tiles per core."""
    T = NB * C
    E_pad = T * 128
    f32, bf16 = mybir.dt.float32, mybir.dt.bfloat16
    X = mybir.AxisListType.X
    MUL, ADD = mybir.AluOpType.mult, mybir.AluOpType.add

    nc = bacc.Bacc("TRN2", target_bir_lowering=False)
    xsT = nc.dram_tensor("xsT", [128, E_pad], bf16, kind="ExternalInput")
    xtT = nc.dram_tensor("xtT", [128, E_pad], bf16, kind="ExternalInput")
    S2 = nc.dram_tensor("S2", [128, T * 128], bf16, kind="ExternalInput")
    OHt = nc.dram_tensor("OHt", [128, T * 4], bf16, kind="ExternalInput")
    Wt = nc.dram_tensor("Wt", [128, 768], bf16, kind="ExternalInput")
    Brep = nc.dram_tensor("Brep", [128, 768], f32, kind="ExternalInput")
    W2ab = nc.dram_tensor("W2ab", [128, 256], bf16, kind="ExternalInput")
    W2c = nc.dram_tensor("W2c", [4, 128], bf16, kind="ExternalInput")
    out = nc.dram_tensor("out", [128, NODES_PAD], f32, kind="ExternalOutput")

    with tile.TileContext(nc) as tc:
        with tc.tile_pool(name="const", bufs=1) as cp, \
             tc.tile_pool(name="io", bufs=2) as iop, \
             tc.tile_pool(name="work", bufs=2) as wp, \
             tc.tile_pool(name="pproj", bufs=1, space="PSUM") as pp, \
             tc.tile_pool(name="pacc", bufs=1, space="PSUM") as pa:

            wt = cp.tile([128, 768], bf16)
            nc.sync.dma_start(out=wt[:], in_=Wt[:, :])
            brep = cp.tile([128, 768], f32)
            nc.sync.dma_start(out=brep[:], in_=Brep[:, :])
            # prime DVE's observed tick for brep's DMA lane so the first
            # bias-add TT needs only its PE wait (TT ISA slot fits 1 wait)
            scratch = cp.tile([128, 1], f32)
            nc.vector.tensor_copy(out=scratch[:], in_=brep[:, 0:1])
            oh = cp.tile([128, T * 4], bf16)
            nc.sync.dma_start(out=oh[:], in_=OHt[:, :])
            w2ab = cp.tile([128, 256], bf16)
            nc.sync.dma_start(out=w2ab[:], in_=W2ab[:, :])
            w2c = cp.tile([4, 128], bf16)
            nc.sync.dma_start(out=w2c[:], in_=W2c[:, :])
            outsb = cp.tile([128, NODES_PAD], f32)

            for b in range(NB):
                esl = slice(b * C * 128, (b + 1) * C * 128)
                xs = iop.tile([128, C * 128], bf16, tag="xs")
                nc.sync.dma_start(out=xs[:], in_=xsT[:, esl])
                xt = iop.tile([128, C * 128], bf16, tag="xt")
                nc.sync.dma_start(out=xt[:], in_=xtT[:, esl])
                sb = iop.tile([128, C * 128], bf16, tag="sb")
                nc.sync.dma_start(out=sb[:], in_=S2[:, esl])

                qkv = wp.tile([128, C * 768], bf16, tag="qkv")
                for i in range(C):
                    ps_q = pp.tile([128, 256], f32, tag="psq")
                    ps_k = pp.tile([128, 256], f32, tag="psk")
                    ps_v = pp.tile([128, 256], f32, tag="psv")
                    ei = slice(i * 128, (i + 1) * 128)
                    nc.tensor.matmul(ps_q[:], lhsT=xs[:, ei],
                                     rhs=wt[:, 0:256], start=True, stop=True)
                    nc.tensor.matmul(ps_k[:], lhsT=xt[:, ei],
                                     rhs=wt[:, 256:512], start=True, stop=True)
                    nc.tensor.matmul(ps_v[:], lhsT=xt[:, ei],
                                     rhs=wt[:, 512:768], start=True, stop=True)
                    # bias add fused with PSUM->SBUF bf16 cast
                    o = i * 768
                    nc.vector.tensor_tensor(
                        out=qkv[:, o:o + 256],
                        in0=ps_q[:], in1=brep[:, 0:256], op=ADD)
                    nc.vector.tensor_tensor(
                        out=qkv[:, o + 256:o + 512],
                        in0=ps_k[:], in1=brep[:, 256:512], op=ADD)
                    nc.vector.tensor_tensor(
                        out=qkv[:, o + 512:o + 768],
                        in0=ps_v[:], in1=brep[:, 512:768], op=ADD)

                # scores: prod[t,h,g,d] = Q[t,h,d] * K[t,g,d]
                # (ISA allows max 3 free dims -> one TT per 128-edge tile)
                prod = wp.tile([128, C * 2048], bf16, tag="prod")
                for i in range(C):
                    o = i * 768
                    qa = (qkv[:, o:o + 256]
                          .rearrange("p (h d) -> p h d", h=H)
                          .unsqueeze(2).to_broadcast([128, H, H, D]))
                    ka = (qkv[:, o + 256:o + 512]
                          .rearrange("p (g d) -> p g d", g=H)
                          .unsqueeze(1).to_broadcast([128, H, H, D]))
                    nc.vector.tensor_tensor(
                        out=prod[:, i * 2048:(i + 1) * 2048]
                            .rearrange("p (h g d) -> p h g d", h=H, g=H),
                        in0=qa, in1=ka, op=MUL)
                scores = wp.tile([128, C * 64], f32, tag="scores")
                nc.vector.tensor_reduce(
                    out=scores[:],
                    in_=prod[:].rearrange("p (a d) -> p a d", d=D),
                    axis=X, op=ADD)
                u = wp.tile([128, C * 64], f32, tag="u")
                nc.scalar.activation(out=u[:], in_=scores[:],
                                     func=mybir.ActivationFunctionType.Exp,
                                     scale=float(1.0 / np.sqrt(D)))
                ssum = wp.tile([128, C * 8], f32, tag="ssum")
                nc.vector.tensor_reduce(
                    out=ssum[:],
                    in_=u[:].rearrange("p (a g) -> p a g", g=H),
                    axis=X, op=ADD)
                rinv = wp.tile([128, C * 8], f32, tag="rinv")
                nc.vector.reciprocal(out=rinv[:], in_=ssum[:])
                attn = wp.tile([128, C * 64], bf16, tag="attn")
                nc.vector.tensor_tensor(
                    out=attn[:].rearrange("p (a g) -> p a g", g=H),
                    in0=u[:].rearrange("p (a g) -> p a g", g=H),
                    in1=rinv[:].rearrange("p a -> p a 1" if False else
                                          "p (a o) -> p a o", o=1)
                        .to_broadcast([128, C * 8, H]),
                    op=MUL)
                # msg[t,h,d] = sum_g attn[t,h,g] * V[t,d,g]  (V host-permuted)
                prod2 = wp.tile([128, C * 2048], bf16, tag="prod")
                for i in range(C):
                    aa = (attn[:, i * 64:(i + 1) * 64]
                          .rearrange("p (h g) -> p h g", h=H)
                          .unsqueeze(2).to_broadcast([128, H, D, H]))
                    va = (qkv[:, i * 768 + 512:(i + 1) * 768]
                          .rearrange("p (d g) -> p d g", d=D)
                          .unsqueeze(1).to_broadcast([128, H, D, H]))
                    nc.vector.tensor_tensor(
                        out=prod2[:, i * 2048:(i + 1) * 2048]
                            .rearrange("p (h d g) -> p h d g", h=H, d=D),
                        in0=aa, in1=va, op=MUL)
                msgf = wp.tile([128, C * 256], f32, tag="msgf")
                nc.vector.tensor_reduce(
                    out=msgf[:],
                    in_=prod2[:].rearrange("p (a g) -> p a g", g=H),
                    axis=X, op=ADD)
                msg = wp.tile([128, C * 256], bf16, tag="msg")
                nc.scalar.activation(out=msg[:], in_=msgf[:],
                                     func=mybir.ActivationFunctionType.Copy)

                # segment sum: aggT = msg_chunk^T @ S  (accumulate over chunks)
                agg1 = pa.tile([128, 128], f32, tag="agg1")
                agg2 = pa.tile([128, 128], f32, tag="agg2")
                agg3 = pa.tile([4, 128], f32, tag="agg3")
                for i in range(C):
                    st, sp = (i == 0), (i == C - 1)
                    s_i = sb[:, i * 128:(i + 1) * 128]
                    nc.tensor.matmul(agg1[:], lhsT=msg[:, i * 256:i * 256 + 128],
                                     rhs=s_i, start=st, stop=sp)
                    nc.tensor.matmul(agg2[:], lhsT=msg[:, i * 256 + 128:(i + 1) * 256],
                                     rhs=s_i, start=st, stop=sp)
                    t_ix = b * C + i
                    nc.tensor.matmul(agg3[:], lhsT=oh[:, t_ix * 4:(t_ix + 1) * 4],
                                     rhs=s_i, start=st, stop=sp)
                a1 = wp.tile([128, 128], bf16, tag="a1")
                nc.scalar.activation(out=a1[:], in_=agg1[:],
                                     func=mybir.ActivationFunctionType.Copy)
                a2 = wp.tile([128, 128], bf16, tag="a2")
                nc.scalar.activation(out=a2[:], in_=agg2[:],
                                     func=mybir.ActivationFunctionType.Copy)
                a3 = wp.tile([4, 128], bf16, tag="a3")
                nc.scalar.activation(out=a3[:], in_=agg3[:],
                                     func=mybir.ActivationFunctionType.Copy)
                mt = pa.tile([128, 128], f32, tag="mt")
                nc.tensor.matmul(mt[:], lhsT=w2ab[:, 0:128], rhs=a1[:],
                                 start=True, stop=False)
                nc.tensor.matmul(mt[:], lhsT=w2ab[:, 128:256], rhs=a2[:],
                                 start=False, stop=False)
                nc.tensor.matmul(mt[:], lhsT=w2c[:], rhs=a3[:],
                                 start=False, stop=True)
                nc.scalar.activation(out=outsb[:, b * 128:(b + 1) * 128],
                                     in_=mt[:],
                                     func=mybir.ActivationFunctionType.Copy)

            nc.sync.dma_start(out=out[:, :], in_=outsb[:])
    return nc


def kernel(node_features, edges, edge_types, Wq, bq, Wk, bk, Wv, bv,
           edge_emb, Wo, bo):
    x = np.asarray(node_features, dtype=np.float32)
    edges = np.asarray(edges, dtype=np.int64)
    et = np.asarray(edge_types, dtype=np.int64)
    Wq = np.asarray(Wq, np.float32); bq = np.asarray(bq, np.float32)
    Wk = np.asarray(Wk, np.float32); bk = np.asarray(bk, np.float32)
    Wv = np.asarray(Wv, np.float32); bv = np.asarray(bv, np.float32)
    edge_emb = np.asarray(edge_emb, np.float32)
    Wo = np.asarray(Wo, np.float32); bo = np.asarray(bo, np.float32)

    src, tgt = edges[:, 0], edges[:, 1]
    core = src // NPC

    # per-core edge lists sorted by local src node, then block-bucketed
    per_core = []
    Cmax = 1
    for c in range(N_CORES):
        idx = np.nonzero(core == c)[0]
        s_loc = src[idx] - c * NPC
        order = np.argsort(s_loc, kind="stable")
        idx, s_loc = idx[order], s_loc[order]
        block = s_loc // 128
        counts = np.bincount(block, minlength=NB)
        Cmax = max(Cmax, int(np.ceil(counts.max() / 128)))
        per_core.append((idx, s_loc, block, counts))
    C = Cmax
    T = NB * C
    E_pad = T * 128

    # shared weight blocks
    Wvp = Wv.reshape(H, D, IN_DIM).transpose(1, 0, 2).reshape(HID, IN_DIM)
    bvp = bv.reshape(H, D).T.reshape(HID)
    Wt_np = np.concatenate([Wq.T, Wk.T, Wvp.T], axis=1).astype(BF16)
    b_t = np.concatenate([bq, bk, bvp]).astype(np.float32)
    Brep_np = np.tile(b_t[None, :], (128, 1)).astype(np.float32)
    WoT = Wo.T.astype(np.float32)
    # W2ab[:, 0:128] = WoT rows 0:128 ; W2ab[:, 128:256] = WoT rows 128:256
    W2ab_np = np.concatenate([WoT[0:128], WoT[128:256]], axis=1).astype(BF16)
    W2c_np = np.concatenate([edge_emb @ Wo.T, bo[None, :]], axis=0).astype(BF16)

    in_maps = []
    for c in range(N_CORES):
        idx, s_loc, block, counts = per_core[c]
        starts = np.zeros(NB, np.int64)
        starts[1:] = np.cumsum(counts)[:-1]
        within = np.arange(len(idx)) - starts[block]
        pos = block * (C * 128) + within

        xs_full = np.zeros((E_pad, IN_DIM), np.float32)
        xs_full[pos] = x[src[idx]]
        xt_full = np.zeros((E_pad, IN_DIM), np.float32)
        xt_full[pos] = x[tgt[idx]]
        S_full = np.zeros((E_pad, 128), np.float32)
        S_full[pos, s_loc - block * 128] = 1.0
        OH_full = np.zeros((E_pad, 4), np.float32)
        OH_full[pos, et[idx]] = 1.0
        OH_full[pos, 3] = 1.0

        in_maps.append({
            "xsT": np.ascontiguousarray(xs_full.T).astype(BF16),
            "xtT": np.ascontiguousarray(xt_full.T).astype(BF16),
            "S2": np.ascontiguousarray(
                S_full.reshape(T, 128, 128).transpose(1, 0, 2)
                .reshape(128, T * 128)).astype(BF16),
            "OHt": np.ascontiguousarray(
                OH_full.reshape(T, 128, 4).transpose(1, 0, 2)
                .reshape(128, T * 4)).astype(BF16),
            "Wt": Wt_np, "Brep": Brep_np,
            "W2ab": W2ab_np, "W2c": W2c_np,
        })

    if C not in _prog_cache:
        nc = _build_program(C)
        nc.finalize()  # runs Bacc passes incl. sync-wait legalization
        _prog_cache[C] = nc
    nc = _prog_cache[C]

    kw = {}
    tr = os.environ.get("KERNEL_TRACE_DIR")
    if tr:
        kw = dict(trace=True, tmpdir=tr)
    res = run_bass_kernel_spmd(nc, in_maps, core_ids=list(range(N_CORES)), **kw)
    global LAST_RESULTS
    LAST_RESULTS = res
    outs = res.results

    messages = np.zeros((N_NODES, IN_DIM), np.float32)
    for c in range(N_CORES):
        o = np.asarray(outs[c]["out"], dtype=np.float32)  # [128, NODES_PAD]
        messages[c * NPC:(c + 1) * NPC, :] = o[:, :NPC].T
    return messages



# revision 10
# speedup vs baseline: 1.1274x; 1.1274x over previous
"""GNN message-passing kernel for Trainium2 (8 NeuronCores, edge-parallel).

Strategy: shard edges by source-node range (host-side stable sort), so each
core owns nodes [c*6250, (c+1)*6250) and exactly the edges rooted there.
Outputs are disjoint -> no collective needed; host concatenates.

Device pipeline per core (E_pad edges in 128-edge tiles, 49 node blocks):
  1. PE: per-edge Q/K/V projections (stationary = gathered-feature tile^T,
     moving = fused [Wq|Wk|Wv'] weight block), PSUM f32.
  2. DVE: bias-add fused into PSUM->SBUF bf16 cast.
  3. DVE/ACT: per-edge 8x8 head attention via broadcast-AP multiply +
     segmented reduce; exp on ACT.
  4. PE: segment-sum via block one-hot matmul (S matrix from host),
     plus edge-type one-hot/count columns for the embedding & bias terms.
  5. PE: final output projection [Wo^T | emb@Wo^T | bo] per node block.
"""

import os
import sys

sys.path.insert(0, "/opt/trn_rl_repo")

import numpy as np
import ml_dtypes

from concourse import bass, bacc, mybir
import concourse.tile as tile
from concourse.bass_utils import run_bass_kernel_spmd

N_NODES = 50000
N_CORES = 8
NPC = N_NODES // N_CORES  # 6250
NB = 49                   # node blocks of 128 per core (49*128 = 6272)
NODES_PAD = NB * 128
IN_DIM = 128
HID = 256
H = 8
D = 32

BF16 = ml_dtypes.bfloat16
_prog_cache = {}
LAST_RESULTS = None


def _build_program(C):
    """C = edge-chunks (of 128) per node block; T = NB*C tiles per core."""
    T = NB * C
    E_pad = T * 128
    f32, bf16 = mybir.dt.float32, mybir.dt.bfloat16
    X = mybir.AxisListType.X
    MUL, ADD = mybir.AluOpType.mult, mybir.AluOpType.add

    nc = bacc.Bacc("TRN2", target_bir_lowering=False)
    xsT = nc.dram_tensor("xsT", [128, E_pad], bf16, kind="ExternalInput")
    xtT = nc.dram_tensor("xtT", [128, E_pad], bf16, kind="ExternalInput")
    S2 = nc.dram_tensor("S2", [128, T * 128], bf16, kind="ExternalInput")
    OHt = nc.dram_tensor("OHt", [128, T * 4], bf16, kind="ExternalInput")
    Wt = nc.dram_tensor("Wt", [128, 768], bf16, kind="ExternalInput")
    Brow = nc.dram_tensor("Brow", [1, 768], bf16, kind="ExternalInput")
    Ones = nc.dram_tensor("Ones", [1, 128], bf16, kind="ExternalInput")
    W2ab = nc.dram_tensor("W2ab", [128, 256], bf16, kind="ExternalInput")
    W2c = nc.dram_tensor("W2c", [4, 128], bf16, kind="ExternalInput")
    out = nc.dram_tensor("out", [128, NODES_PAD], f32, kind="ExternalOutput")

    with tile.TileContext(nc) as tc:
        with tc.tile_pool(name="const", bufs=1) as cp, \
             tc.tile_pool(name="io", bufs=2) as iop, \
             tc.tile_pool(name="work", bufs=2) as wp, \
             tc.tile_pool(name="pproj", bufs=1, space="PSUM") as pp, \
             tc.tile_pool(name="pacc", bufs=1, space="PSUM") as pa:

            wt = cp.tile([128, 768], bf16)
            nc.sync.dma_start(out=wt[:], in_=Wt[:, :])
            brow = cp.tile([1, 768], bf16)
            nc.sync.dma_start(out=brow[:], in_=Brow[:, :])
            ones = cp.tile([1, 128], bf16)
            nc.sync.dma_start(out=ones[:], in_=Ones[:, :])
            oh = cp.tile([128, T * 4], bf16)
            nc.sync.dma_start(out=oh[:], in_=OHt[:, :])
            w2ab = cp.tile([128, 256], bf16)
            nc.sync.dma_start(out=w2ab[:], in_=W2ab[:, :])
            w2c = cp.tile([4, 128], bf16)
            nc.sync.dma_start(out=w2c[:], in_=W2c[:, :])
            outsb = cp.tile([128, NODES_PAD], f32)

            for b in range(NB):
                esl = slice(b * C * 128, (b + 1) * C * 128)
                xs = iop.tile([128, C * 128], bf16, tag="xs")
                nc.sync.dma_start(out=xs[:], in_=xsT[:, esl])
                xt = iop.tile([128, C * 128], bf16, tag="xt")
                nc.sync.dma_start(out=xt[:], in_=xtT[:, esl])
                sb = iop.tile([128, C * 128], bf16, tag="sb")
                nc.sync.dma_start(out=sb[:], in_=S2[:, esl])

                qkv = wp.tile([128, C * 768], bf16, tag="qkv")
                for i in range(C):
                    ps_q = pp.tile([128, 256], f32, tag="psq")
                    ps_k = pp.tile([128, 256], f32, tag="psk")
                    ps_v = pp.tile([128, 256], f32, tag="psv")
                    ei = slice(i * 128, (i + 1) * 128)
                    # bias via rank-1 PSUM seed (ones^T x brow), then the
                    # projection accumulates on top -> bias-add is free on PE
                    nc.tensor.matmul(ps_q[:], lhsT=ones[:],
                                     rhs=brow[:, 0:256], start=True, stop=False)
                    nc.tensor.matmul(ps_q[:], lhsT=xs[:, ei],
                                     rhs=wt[:, 0:256], start=False, stop=True)
                    nc.tensor.matmul(ps_k[:], lhsT=ones[:],
                                     rhs=brow[:, 256:512], start=True, stop=False)
                    nc.tensor.matmul(ps_k[:], lhsT=xt[:, ei],
                                     rhs=wt[:, 256:512], start=False, stop=True)
                    nc.tensor.matmul(ps_v[:], lhsT=ones[:],
                                     rhs=brow[:, 512:768], start=True, stop=False)
                    nc.tensor.matmul(ps_v[:], lhsT=xt[:, ei],
                                     rhs=wt[:, 512:768], start=False, stop=True)
                    # PSUM->SBUF bf16 cast on the scalar engine (off DVE)
                    o = i * 768
                    nc.scalar.activation(
                        out=qkv[:, o:o + 256], in_=ps_q[:],
                        func=mybir.ActivationFunctionType.Copy)
                    nc.scalar.activation(
                        out=qkv[:, o + 256:o + 512], in_=ps_k[:],
                        func=mybir.ActivationFunctionType.Copy)
                    nc.scalar.activation(
                        out=qkv[:, o + 512:o + 768], in_=ps_v[:],
                        func=mybir.ActivationFunctionType.Copy)

                # scores: prod[t,h,g,d] = Q[t,h,d] * K[t,g,d]
                # (ISA allows max 3 free dims -> one TT per 128-edge tile)
                prod = wp.tile([128, C * 2048], bf16, tag="prod")
                for i in range(C):
                    o = i * 768
                    qa = (qkv[:, o:o + 256]
                          .rearrange("p (h d) -> p h d", h=H)
                          .unsqueeze(2).to_broadcast([128, H, H, D]))
                    ka = (qkv[:, o + 256:o + 512]
                          .rearrange("p (g d) -> p g d", g=H)
                          .unsqueeze(1).to_broadcast([128, H, H, D]))
                    nc.vector.tensor_tensor(
                        out=prod[:, i * 2048:(i + 1) * 2048]
                            .rearrange("p (h g d) -> p h g d", h=H, g=H),
                        in0=qa, in1=ka, op=MUL)
                scores = wp.tile([128, C * 64], bf16, tag="scores")
                with nc.allow_low_precision(reason="bf16 scores, fp32 accum"):
                    nc.vector.tensor_reduce(
                        out=scores[:],
                        in_=prod[:].rearrange("p (a d) -> p a d", d=D),
                        axis=X, op=ADD)
                u = wp.tile([128, C * 64], bf16, tag="u")
                nc.scalar.activation(out=u[:], in_=scores[:],
                                     func=mybir.ActivationFunctionType.Exp,
                                     scale=float(1.0 / np.sqrt(D)))
                ssum = wp.tile([128, C * 8], bf16, tag="ssum")
                rinv = wp.tile([128, C * 8], bf16, tag="rinv")
                with nc.allow_low_precision(reason="softmax denom bf16"):
                    nc.vector.tensor_reduce(
                        out=ssum[:],
                        in_=u[:].rearrange("p (a g) -> p a g", g=H),
                        axis=X, op=ADD)
                    nc.vector.reciprocal(out=rinv[:], in_=ssum[:])
                attn = wp.tile([128, C * 64], bf16, tag="attn")
                nc.vector.tensor_tensor(
                    out=attn[:].rearrange("p (a g) -> p a g", g=H),
                    in0=u[:].rearrange("p (a g) -> p a g", g=H),
                    in1=rinv[:].rearrange("p (a o) -> p a o", o=1)
                        .to_broadcast([128, C * 8, H]),
                    op=MUL)
                # msg[t,h,d] = sum_g attn[t,h,g] * V[t,d,g]  (V host-permuted)
                prod2 = wp.tile([128, C * 2048], bf16, tag="prod")
                for i in range(C):
                    aa = (attn[:, i * 64:(i + 1) * 64]
                          .rearrange("p (h g) -> p h g", h=H)
                          .unsqueeze(2).to_broadcast([128, H, D, H]))
                    va = (qkv[:, i * 768 + 512:(i + 1) * 768]
                          .rearrange("p (d g) -> p d g", d=D)
                          .unsqueeze(1).to_broadcast([128, H, D, H]))
                    nc.vector.tensor_tensor(
                        out=prod2[:, i * 2048:(i + 1) * 2048]
                            .rearrange("p (h d g) -> p h d g", h=H, d=D),
                        in0=aa, in1=va, op=MUL)
                msg = wp.tile([128, C * 256], bf16, tag="msg")
                with nc.allow_low_precision(reason="bf16 msg, fp32 accum"):
                    nc.vector.tensor_reduce(
                        out=msg[:],
                        in_=prod2[:].rearrange("p (a g) -> p a g", g=H),
                        axis=X, op=ADD)

                # segment sum: aggT = msg_chunk^T @ S  (accumulate over chunks)
                agg1 = pa.tile([128, 128], f32, tag="agg1")
                agg2 = pa.tile([128, 128], f32, tag="agg2")
                agg3 = pa.tile([4, 128], f32, tag="agg3")
                for i in range(C):
                    st, sp = (i == 0), (i == C - 1)
                    s_i = sb[:, i * 128:(i + 1) * 128]
                    nc.tensor.matmul(agg1[:], lhsT=msg[:, i * 256:i * 256 + 128],
                                     rhs=s_i, start=st, stop=sp)
                    nc.tensor.matmul(agg2[:], lhsT=msg[:, i * 256 + 128:(i + 1) * 256],
                                     rhs=s_i, start=st, stop=sp)
                    t_ix = b * C + i
                    nc.tensor.matmul(agg3[:], lhsT=oh[:, t_ix * 4:(t_ix + 1) * 4],
                                     rhs=s_i, start=st, stop=sp)
                a1 = wp.tile([128, 128], bf16, tag="a1")
                nc.scalar.activation(out=a1[:], in_=agg1[:],
                                     func=mybir.ActivationFunctionType.Copy)
                a2 = wp.tile([128, 128], bf16, tag="a2")
                nc.scalar.activation(out=a2[:], in_=agg2[:],
                                     func=mybir.ActivationFunctionType.Copy)
                a3 = wp.tile([4, 128], bf16, tag="a3")
                nc.scalar.activation(out=a3[:], in_=agg3[:],
                                     func=mybir.ActivationFunctionType.Copy)
                mt = pa.tile([128, 128], f32, tag="mt")
                nc.tensor.matmul(mt[:], lhsT=w2ab[:, 0:128], rhs=a1[:],
                                 start=True, stop=False)
                nc.tensor.matmul(mt[:], lhsT=w2ab[:, 128:256], rhs=a2[:],
                                 start=False, stop=False)
                nc.tensor.matmul(mt[:], lhsT=w2c[:], rhs=a3[:],
                                 start=False, stop=True)
                nc.scalar.activation(out=outsb[:, b * 128:(b + 1) * 128],
                                     in_=mt[:],
                                     func=mybir.ActivationFunctionType.Copy)

            nc.sync.dma_start(out=out[:, :], in_=outsb[:])
    return nc


def kernel(node_features, edges, edge_types, Wq, bq, Wk, bk, Wv, bv,
           edge_emb, Wo, bo):
    x = np.asarray(node_features, dtype=np.float32)
    edges = np.asarray(edges, dtype=np.int64)
    et = np.asarray(edge_types, dtype=np.int64)
    Wq = np.asarray(Wq, np.float32); bq = np.asarray(bq, np.float32)
    Wk = np.asarray(Wk, np.float32); bk = np.asarray(bk, np.float32)
    Wv = np.asarray(Wv, np.float32); bv = np.asarray(bv, np.float32)
    edge_emb = np.asarray(edge_emb, np.float32)
    Wo = np.asarray(Wo, np.float32); bo = np.asarray(bo, np.float32)

    src, tgt = edges[:, 0], edges[:, 1]
    core = src // NPC

    # per-core edge lists sorted by local src node, then block-bucketed
    per_core = []
    Cmax = 1
    for c in range(N_CORES):
        idx = np.nonzero(core == c)[0]
        s_loc = src[idx] - c * NPC
        order = np.argsort(s_loc, kind="stable")
        idx, s_loc = idx[order], s_loc[order]
        block = s_loc // 128
        counts = np.bincount(block, minlength=NB)
        Cmax = max(Cmax, int(np.ceil(counts.max() / 128)))
        per_core.append((idx, s_loc, block, counts))
    C = Cmax
    T = NB * C
    E_pad = T * 128

    # shared weight blocks
    Wvp = Wv.reshape(H, D, IN_DIM).transpose(1, 0, 2).reshape(HID, IN_DIM)
    bvp = bv.reshape(H, D).T.reshape(HID)
    Wt_np = np.concatenate([Wq.T, Wk.T, Wvp.T], axis=1).astype(BF16)
    b_t = np.concatenate([bq, bk, bvp]).astype(np.float32)
    Brow_np = b_t[None, :].astype(BF16)
    Ones_np = np.ones((1, 128), dtype=BF16)
    WoT = Wo.T.astype(np.float32)
    # W2ab[:, 0:128] = WoT rows 0:128 ; W2ab[:, 128:256] = WoT rows 128:256
    W2ab_np = np.concatenate([WoT[0:128], WoT[128:256]], axis=1).astype(BF16)
    W2c_np = np.concatenate([edge_emb @ Wo.T, bo[None, :]], axis=0).astype(BF16)

    in_maps = []
    for c in range(N_CORES):
        idx, s_loc, block, counts = per_core[c]
        starts = np.zeros(NB, np.int64)
        starts[1:] = np.cumsum(counts)[:-1]
        within = np.arange(len(idx)) - starts[block]
        pos = block * (C * 128) + within

        xs_full = np.zeros((E_pad, IN_DIM), np.float32)
        xs_full[pos] = x[src[idx]]
        xt_full = np.zeros((E_pad, IN_DIM), np.float32)
        xt_full[pos] = x[tgt[idx]]
        S_full = np.zeros((E_pad, 128), np.float32)
        S_full[pos, s_loc - block * 128] = 1.0
        OH_full = np.zeros((E_pad, 4), np.float32)
        OH_full[pos, et[idx]] = 1.0
        OH_full[pos, 3] = 1.0

        in_maps.append({
            "xsT": np.ascontiguousarray(xs_full.T).astype(BF16),
            "xtT": np.ascontiguousarray(xt_full.T).astype(BF16),
            "S2": np.ascontiguousarray(
                S_full.reshape(T, 128, 128).transpose(1, 0, 2)
                .reshape(128, T * 128)).astype(BF16),
            "OHt": np.ascontiguousarray(
                OH_full.reshape(T, 128, 4).transpose(1, 0, 2)
                .reshape(128, T * 4)).astype(BF16),
            "Wt": Wt_np, "Brow": Brow_np, "Ones": Ones_np,
            "W2ab": W2ab_np, "W2c": W2c_np,
        })

    if C not in _prog_cache:
        nc = _build_program(C)
        nc.finalize()  # runs Bacc passes incl. sync-wait legalization
        _prog_cache[C] = nc
    nc = _prog_cache[C]

    kw = {}
    tr = os.environ.get("KERNEL_TRACE_DIR")
    if tr:
        kw = dict(trace=True, tmpdir=tr)
    res = run_bass_kernel_spmd(nc, in_maps, core_ids=list(range(N_CORES)), **kw)
    global LAST_RESULTS
    LAST_RESULTS = res
    outs = res.results

    messages = np.zeros((N_NODES, IN_DIM), np.float32)
    for c in range(N_CORES):
        o = np.asarray(outs[c]["out"], dtype=np.float32)  # [128, NODES_PAD]
        messages[c * NPC:(c + 1) * NPC, :] = o[:, :NPC].T
    return messages

